# revision 13
# baseline (speedup 1.0000x reference)
"""Fused LSTM-cell kernel for 8x Trainium2 NeuronCores (Bass/Tile).

Strategy: data-parallel over the batch. Each of the 8 cores handles 512
batch rows and computes all gates over the full hidden dim:

    gates[b, g, h] = x[b,:] @ W[g, h, :] + h_prev[b,:] @ V[g, h, :] + bias[g, h]

The two GEMMs are fused into one K=4096 contraction by concatenating
A = [x | h_prev] and stacking Wf = [W^T; V^T] (shared by all cores).
The 8192 fused output columns are reordered into 16 slabs of 512 where a
slab holds all 4 gates for 128 hidden columns — so each PSUM tile can be
combined into h_next/c_next immediately. Weights stream slab-by-slab
(~146 GB/s demand, well under HBM bandwidth), so the PE never waits on a
front-loaded weight burst. Inputs are cast to bf16 on the host (PSUM
accumulation stays fp32); gate math runs in fp32 on ACT/DVE.
"""

import sys
import numpy as np

for _p in ("/opt/trn_rl_repo", "/root/.axon_site/_ro/trn_rl_repo"):
    if _p not in sys.path:
        sys.path.insert(0, _p)

import ml_dtypes

B = 4096
I_DIM = 2048
H_DIM = 2048
G = 4
N_CORES = 8
BS = B // N_CORES              # 512 batch rows per core
MT = BS // 128                 # 4 m-tiles per core
K_TOT = I_DIM + H_DIM          # 4096 fused contraction
KT = K_TOT // 128              # 32 k-tiles
HB = 128                       # hidden columns per slab
S = H_DIM // HB                # 16 slabs
SLAB_N = G * HB                # 512 output columns per slab (PSUM bank)
W_DMA_CHUNK = 8                # k-tiles per weight DMA (8*512*2B*128 = 1MB)
MM_DTYPE = "fp16"              # "fp16" | "bf16": fp16 is same PE speed, 8x accuracy

_COMPILED = None
TRACE = False          # test harness sets True to capture an NTFF profile
LAST_EXEC_NS = None
LAST_RESULT = None


def _build_program():
    import concourse.mybir as mybir
    import concourse.tile as tile
    from concourse import bacc

    dt = mybir.dt
    mm_dt = dt.float16 if MM_DTYPE == "fp16" else dt.bfloat16
    nc = bacc.Bacc("TRN2", target_bir_lowering=False, debug=False,
                   num_devices=N_CORES)

    a_dram = nc.dram_tensor("a_t", [MT, 128, K_TOT], mm_dt,
                            kind="ExternalInput").ap()
    w_dram = nc.dram_tensor("w_sl", [S, 128, KT, SLAB_N], mm_dt,
                            kind="ExternalInput").ap()
    bias_dram = nc.dram_tensor("bias_sl", [S, 128, SLAB_N], dt.float32,
                               kind="ExternalInput").ap()
    cprev_dram = nc.dram_tensor("c_prev_s", [BS, H_DIM], dt.float32,
                                kind="ExternalInput").ap()
    h_out = nc.dram_tensor("h_out", [BS, H_DIM], dt.float32,
                           kind="ExternalOutput").ap()
    c_out = nc.dram_tensor("c_out", [BS, H_DIM], dt.float32,
                           kind="ExternalOutput").ap()

    SIG = mybir.ActivationFunctionType.Sigmoid
    TANH = mybir.ActivationFunctionType.Tanh

    with tile.TileContext(nc) as tc:
        with (
            tc.tile_pool(name="apool", bufs=1) as apool,
            tc.tile_pool(name="wpool", bufs=2) as wpool,
            tc.tile_pool(name="bpool", bufs=2) as bpool,
            tc.tile_pool(name="cppool", bufs=4) as cppool,
            tc.tile_pool(name="psum", bufs=4, space="PSUM") as pspool,
            tc.tile_pool(name="gpool", bufs=2) as gpool,
            tc.tile_pool(name="actpool", bufs=2) as actpool,
            tc.tile_pool(name="tpool", bufs=2) as tpool,
            tc.tile_pool(name="opool", bufs=4) as opool,
        ):
            # Activations resident in SBUF. Interleave the leading a/w DMAs in
            # small chunks so the very first matmuls (m=0, low kt) have their
            # inputs a few microseconds earlier.
            a_all = apool.tile([128, MT, K_TOT], mm_dt, tag="a_all")
            w_first = wpool.tile([128, KT, SLAB_N], mm_dt, tag="w_sb")
            AC = K_TOT // 4    # a chunk: 8 k-tiles worth of columns
            for q in range(4):
                nc.sync.dma_start(a_all[:, 0, q * AC:(q + 1) * AC],
                                  a_dram[0][:, q * AC:(q + 1) * AC])
                kc = q * 2
                nc.sync.dma_start(w_first[:, kc:kc + 2, :],
                                  w_dram[0, :, kc:kc + 2, :])
            for kc in range(8, KT, W_DMA_CHUNK):
                nc.sync.dma_start(w_first[:, kc:kc + W_DMA_CHUNK, :],
                                  w_dram[0, :, kc:kc + W_DMA_CHUNK, :])
            for m in range(1, MT):
                nc.sync.dma_start(a_all[:, m, :], a_dram[m])

            for s in range(S):
                if s == 0:
                    w_sb = w_first
                else:
                    w_sb = wpool.tile([128, KT, SLAB_N], mm_dt, tag="w_sb")
                    for kc in range(0, KT, W_DMA_CHUNK):
                        nc.sync.dma_start(w_sb[:, kc:kc + W_DMA_CHUNK, :],
                                          w_dram[s, :, kc:kc + W_DMA_CHUNK, :])
                bias_sb = bpool.tile([128, SLAB_N], dt.float32, tag="bias_sb")
                nc.sync.dma_start(bias_sb[:], bias_dram[s])

                for m in range(MT):
                    cp_sb = cppool.tile([128, HB], dt.float32, tag="cp_sb")
                    nc.sync.dma_start(
                        cp_sb[:],
                        cprev_dram[m * 128:(m + 1) * 128,
                                   s * HB:(s + 1) * HB])

                    ps = pspool.tile([128, SLAB_N], dt.float32, tag="ps")
                    for kt in range(KT):
                        nc.tensor.matmul(
                            ps[:],
                            a_all[:, m, kt * 128:(kt + 1) * 128],
                            w_sb[:, kt, :],
                            start=(kt == 0),
                            stop=(kt == KT - 1),
                        )

                    # PSUM eviction fused with the per-column bias add
                    g_sb = gpool.tile([128, SLAB_N], dt.float32, tag="g_sb")
                    nc.vector.tensor_add(g_sb[:], ps[:], bias_sb[:])
                    acts = actpool.tile([128, SLAB_N], dt.float32, tag="acts")
                    nc.scalar.activation(acts[:, 0:3 * HB],
                                         g_sb[:, 0:3 * HB], SIG)
                    nc.scalar.activation(acts[:, 3 * HB:4 * HB],
                                         g_sb[:, 3 * HB:4 * HB], TANH)

                    t0 = tpool.tile([128, HB], dt.float32, tag="t0")
                    nc.vector.tensor_mul(t0[:], acts[:, 0:HB], cp_sb[:])
                    t1 = tpool.tile([128, HB], dt.float32, tag="t1")
                    nc.vector.tensor_mul(t1[:], acts[:, HB:2 * HB],
                                         acts[:, 3 * HB:4 * HB])
                    c_t = opool.tile([128, HB], dt.float32, tag="c_t")
                    nc.vector.tensor_add(c_t[:], t0[:], t1[:])
                    th = tpool.tile([128, HB], dt.float32, tag="th")
                    nc.scalar.activation(th[:], c_t[:], TANH)
                    h_t = opool.tile([128, HB], dt.float32, tag="h_t")
                    nc.vector.tensor_mul(h_t[:], acts[:, 2 * HB:3 * HB],
                                         th[:])

                    nc.sync.dma_start(
                        c_out[m * 128:(m + 1) * 128, s * HB:(s + 1) * HB],
                        c_t[:])
                    nc.sync.dma_start(
                        h_out[m * 128:(m + 1) * 128, s * HB:(s + 1) * HB],
                        h_t[:])

    nc.compile()
    return nc


def _prep_inputs(x, h_prev, c_prev, W, bW, V, bV, b):
    bf16 = np.float16 if MM_DTYPE == "fp16" else ml_dtypes.bfloat16
    x = np.asarray(x, np.float32)
    h_prev = np.asarray(h_prev, np.float32)
    c_prev = np.asarray(c_prev, np.float32)
    W = np.asarray(W, np.float32)
    bW = np.asarray(bW, np.float32)
    V = np.asarray(V, np.float32)
    bV = np.asarray(bV, np.float32)
    b = np.asarray(b, np.float32)

    A = np.concatenate([x, h_prev], axis=1).astype(bf16)        # [B, K]

    # Fused weights, shared by all cores.
    # w_sl[s, p, kt, g*HB + jj] = WV[g, s*HB + jj, kt*128 + p]
    WV = np.concatenate([W, V], axis=2).astype(bf16)            # [G, H, K]
    w_sl = np.ascontiguousarray(
        WV.reshape(G, S, HB, KT, 128).transpose(1, 4, 3, 0, 2)
    ).reshape(S, 128, KT, SLAB_N)

    bias_full = (bW + bV + b).astype(np.float32)                # [G, H]
    # bias_sl[s, p, g*HB + jj] = bias_full[g, s*HB + jj]
    bias_row = bias_full.reshape(G, S, HB).transpose(1, 0, 2).reshape(S, SLAB_N)
    bias_sl = np.ascontiguousarray(
        np.broadcast_to(bias_row[:, None, :], (S, 128, SLAB_N)))

    in_maps = []
    for c in range(N_CORES):
        r0, r1 = c * BS, (c + 1) * BS
        # a_t[m, p, kt*128 + j] = A[r0 + m*128 + j, kt*128 + p]
        a_t = np.ascontiguousarray(
            A[r0:r1].reshape(MT, 128, KT, 128).transpose(0, 3, 2, 1)
        ).reshape(MT, 128, K_TOT)
        in_maps.append({
            "a_t": a_t,
            "w_sl": w_sl,
            "bias_sl": bias_sl,
            "c_prev_s": np.ascontiguousarray(c_prev[r0:r1]),
        })
    return in_maps


def kernel(x, h_prev, c_prev, W, bW, V, bV, b):
    global _COMPILED
    from concourse.bass_utils import run_bass_kernel_spmd

    if _COMPILED is None:
        _COMPILED = _build_program()
    nc = _COMPILED

    in_maps = _prep_inputs(x, h_prev, c_prev, W, bW, V, bV, b)
    res = run_bass_kernel_spmd(nc, in_maps, list(range(N_CORES)), trace=TRACE)
    global LAST_EXEC_NS, LAST_RESULT
    LAST_EXEC_NS = res.exec_time_ns
    LAST_RESULT = res

    h_next = np.concatenate([res.results[c]["h_out"] for c in range(N_CORES)],
                            axis=0)
    c_next = np.concatenate([res.results[c]["c_out"] for c in range(N_CORES)],
                            axis=0)
    return (h_next, c_next)
